# revision 20
# baseline (speedup 1.0000x reference)
"""LocallyConnected1D Trainium2 kernel (8-core SPMD, Bass/Tile).

out[b,o,l] = sum_{i,k} x[b,i,l+k] * w[l,o,i,k] + bias[o,l]
  B=64, I=O=128, K=8, L_in=512, L_out=505 (stride 1), fp32 I/O.

Sharding: OUT_LEN across 8 cores (64 positions each, padded 505->512).
Each position is an independent GEMM: out[:, :, l] = X_l @ W_l with
contract dim I*K=1024 split into 8 accumulating 128-contract matmuls.
Weight slice [i, o] is the stationary operand (full 128x128 array),
x window [i, b] streams.

Precision: weights and x are cast to fp8 e3m4 on host (the weight DMA
is the roofline: 265MB fp32 -> 66MB fp8), PSUM accumulates fp32, bias
is added in fp32 on DVE, and the output is written back bf16 and
upcast to fp32 on host. Measured end-to-end rel err 1.68e-2 (L2) /
1.77e-2 (max, absmax-scaled) on the fixed-seed reference inputs —
under the 2e-2 gate; set x_fp8=False (bf16 x, 33.8us) for 1.20e-2.

Schedule (tuned against TimelineSim, 98%+ DMA-pool occupancy):
- weight blocks taper small at BOTH ends ((2,2,4)+(8,)*6+(4,2,1,1)):
  small head blocks start the PE early; small tail blocks shorten the
  last w-arrival -> compute -> final-out drain chain.
- x arrives in column chunks interleaved with the weight stream.
- out blocks (48,8,4,4): one big mid-stream writeback + tapered tail,
  so most out bytes ride the DMA pool during the tail drain instead of
  delaying the weight stream.
- DMA queue split: w/x on SP (HWDGE), x0/bias/outs on Pool (SWDGE),
  final out on Activation — dependency waits on an in-order sequencer
  must not stall the weight stream behind them.
"""

import json

import numpy as np
import ml_dtypes

B = 64
IC = 128
OC = 128
KW = 8
LIN = 512
LOUT = 505
NCORES = 8
LPC = 64  # padded positions per core: 8*64 = 512 >= 505
TW = LPC + KW - 1  # x time-columns a core touches (71)
TPAD = (NCORES - 1) * LPC + TW  # padded x length (519)
OB = 8  # x-chunk width (columns) and w/out block alignment granularity

_BF16 = ml_dtypes.bfloat16
_F8 = ml_dtypes.float8_e3m4

_CACHE: dict = {}
LAST_RESULTS = None  # BassKernelResults of the most recent kernel() call


# --- workaround: this walrus build rejects >1 sync wait per instruction ----
def _split_waits(raw: bytes) -> bytes:
    m = json.loads(raw)
    ctr = 0
    for f in m.get("functions", []):
        for blk in f.get("blocks", []) or f.get("basicblocks", []):
            out = []
            for inst in blk.get("instructions", []):
                si = inst.get("sync_info")
                waits = (si or {}).get("on_wait") or []
                if len(waits) > 1:
                    for w in waits[:-1]:
                        ctr += 1
                        out.append(
                            {
                                "debug": inst.get("debug", 0),
                                "engine": inst["engine"],
                                "ins": [],
                                "name": f"waitsplit_{ctr}",
                                "opcode": "EventSemaphore",
                                "outs": [],
                                "sync_info": {"on_update": [], "on_wait": [w]},
                            }
                        )
                    si["on_wait"] = waits[-1:]
                out.append(inst)
            blk["instructions"] = out
    return json.dumps(m).encode()


def _build_bass(w_bufs: int = 3, psum_bufs: int = 8, out_bufs: int = 3,
                w_sched=(2, 2, 4) + (8,) * 6 + (4, 2, 1, 1),
                out_sched=(48, 8, 4, 4),
                xa_cols: int = 16, out_eng: str = "gpsimd",
                bias_eng: str = "gpsimd", x0_eng: str = "gpsimd",
                x_eng: str = "sync", final_out_eng: str | None = "scalar",
                x_fp8: bool = True):
    import concourse.bass as bass
    import concourse.tile as tile
    import concourse.mybir as mybir

    sched = list(w_sched)
    assert sum(sched) == LPC
    osched = list(out_sched)
    assert sum(osched) == LPC
    # w blocks must not straddle out blocks
    obounds = [0]
    for nb in osched:
        obounds.append(obounds[-1] + nb)
    acc = 0
    for nb in sched:
        assert any(a <= acc and acc + nb <= b
                   for a, b in zip(obounds[:-1], obounds[1:]))
        acc += nb

    # x column chunks: [0, xa_cols) then OB-wide chunks to TW
    xbounds = [0, xa_cols]
    while xbounds[-1] < TW:
        xbounds.append(min(xbounds[-1] + OB, TW))

    xdt = mybir.dt.float8e3 if x_fp8 else mybir.dt.bfloat16

    nc = bass.Bass()
    x_d = nc.dram_tensor("x", [IC, TW, B], xdt, kind="ExternalInput")
    w_d = nc.dram_tensor(
        "w", [IC, LPC, KW, OC], mybir.dt.float8e3, kind="ExternalInput"
    )
    b_d = nc.dram_tensor("bias", [OC, LPC], mybir.dt.float32, kind="ExternalInput")
    o_d = nc.dram_tensor("out", [OC, LPC, B], mybir.dt.bfloat16, kind="ExternalOutput")

    # out DMAs go on their own queue: their compute-dependency waits must not
    # block later weight-block DMAs behind them on SP's in-order sequencer
    oeng = getattr(nc, out_eng)

    with tile.TileContext(nc) as tc:
        with (
            tc.tile_pool(name="const", bufs=1) as constp,
            tc.tile_pool(name="wp", bufs=w_bufs) as wp,
            tc.tile_pool(name="op", bufs=out_bufs) as op,
            tc.tile_pool(name="ps", bufs=psum_bufs, space="PSUM") as pp,
        ):
            # x chunk tiles; chunk 0 lands first so the PE can start early
            xtiles = []  # (start_col, tile)
            nchunks = len(xbounds) - 1
            for ci in range(nchunks):
                c0, c1 = xbounds[ci], xbounds[ci + 1]
                xt = constp.tile([IC, c1 - c0, B], xdt,
                                 name=f"x{ci}", tag=f"x{ci}")
                xtiles.append((c0, xt))
            xdma_done = [False] * nchunks

            def need_x(col):
                ci = next(i for i in range(nchunks)
                          if xbounds[i] <= col < xbounds[i + 1])
                if not xdma_done[ci]:
                    c0, xt = xtiles[ci]
                    eng = getattr(nc, x0_eng if ci == 0 else x_eng)
                    eng.dma_start(xt[:], x_d[:, c0: c0 + xt.shape[1]])
                    xdma_done[ci] = True
                return ci

            def x_ap(col):
                ci = need_x(col)
                c0, xt = xtiles[ci]
                return xt[:, col - c0, :]

            need_x(0)
            bt = constp.tile([OC, LPC], mybir.dt.float32)
            getattr(nc, bias_eng).dma_start(bt[:], b_d[:])

            blocks = []  # (l0, nb)
            l0 = 0
            for nb in sched:
                blocks.append((l0, nb))
                l0 += nb
            bi = 0  # next block to process
            wt = None
            wl0 = wnb = 0

            for ol0, onb in zip(obounds[:-1], osched):
                ot = op.tile([OC, onb, B], mybir.dt.bfloat16,
                             name=f"ot{onb}", tag=f"ot{onb}")
                for j in range(onb):
                    l = ol0 + j
                    if wt is None or l >= wl0 + wnb:
                        wl0, wnb = blocks[bi]
                        bi += 1
                        wt = wp.tile([IC, wnb, KW, OC], mybir.dt.float8e3,
                                     name=f"wt{wnb}", tag=f"wt{wnb}")
                        # prefetch all x chunks this block touches, then weights
                        need_x(wl0 + wnb - 1 + KW - 1)
                        nc.sync.dma_start(wt[:], w_d[:, wl0: wl0 + wnb])
                    ps = pp.tile([OC, B], mybir.dt.float32)
                    for k in range(KW):
                        nc.tensor.matmul(
                            ps[:],
                            wt[:, l - wl0, k, :],
                            x_ap(l + k),
                            start=(k == 0),
                            stop=(k == KW - 1),
                        )
                    nc.vector.tensor_scalar_add(
                        ot[:, j, :], ps[:], bt[:, l: l + 1]
                    )
                eng = oeng
                if final_out_eng is not None and ol0 + onb == LPC:
                    eng = getattr(nc, final_out_eng)
                eng.dma_start(o_d[:, ol0: ol0 + onb, :], ot[:])

    fixed = _split_waits(bass.Bass.to_json_bytes(nc))
    nc.to_json_bytes = lambda: fixed  # type: ignore[method-assign]
    return nc


def _prepare_inputs(x, weight, bias, x_fp8=True):
    x = np.asarray(x, dtype=np.float32)
    weight = np.asarray(weight, dtype=np.float32)
    bias = np.asarray(bias, dtype=np.float32)

    # x: [b, i, t] -> bf16/fp8, pad t to TPAD, transpose -> [i, t, b]
    xdt = _F8 if x_fp8 else _BF16
    xpad = np.zeros((B, IC, TPAD), dtype=xdt)
    xpad[:, :, :LIN] = x.astype(xdt)
    xt = xpad.transpose(1, 2, 0)  # [i, t, b] view

    # weight: [l, o, i, k] -> fp8 e3m4, pad l, transpose -> [i, l, k, o]
    wpad = np.zeros((NCORES * LPC, OC, IC, KW), dtype=_F8)
    wpad[:LOUT] = weight.astype(_F8)
    wt = wpad.transpose(2, 0, 3, 1)  # [i, l, k, o] view

    bpad = np.zeros((OC, NCORES * LPC), dtype=np.float32)
    bpad[:, :LOUT] = bias

    in_maps = []
    for c in range(NCORES):
        l0 = c * LPC
        in_maps.append(
            {
                "x": np.ascontiguousarray(xt[:, l0: l0 + TW, :]),
                "w": np.ascontiguousarray(wt[:, l0: l0 + LPC]),
                "bias": np.ascontiguousarray(bpad[:, l0: l0 + LPC]),
            }
        )
    return in_maps


def _assemble(results):
    full = np.stack([results[c]["out"] for c in range(NCORES)], axis=0)
    # [c, o, l_loc, b] (bf16) -> fp32 [b, o, c*LPC + l_loc] -> crop to LOUT
    out = (
        full.astype(np.float32)
        .transpose(3, 1, 0, 2)
        .reshape(B, OC, NCORES * LPC)[:, :, :LOUT]
    )
    return np.ascontiguousarray(out)


def kernel(x, weight, bias):
    global LAST_RESULTS
    from concourse.bass_utils import run_bass_kernel_spmd

    if "nc" not in _CACHE:
        _CACHE["nc"] = _build_bass()
    nc = _CACHE["nc"]
    in_maps = _prepare_inputs(x, weight, bias)
    res = run_bass_kernel_spmd(nc, in_maps, core_ids=list(range(NCORES)))
    LAST_RESULTS = res
    return _assemble(res.results)
